# revision 15
# baseline (speedup 1.0000x reference)
"""Criss-cross attention (CCAttention) Trainium2 kernel.

Shapes (hardcoded): x [8, 288, 128, 128] f32, Wq/Wk [36, 288], Wv [288, 288],
bq/bk [36], bv [288], eca_w [3], gamma [1]. Output [8, 288, 128, 128] f32.

Sharding: pure data parallel — one batch element per NeuronCore (8 cores).

Per-core algorithm (batch index dropped):
  q/k/v are 1x1 convs (channel GEMMs). Column attention couples pixels that
  share w; row attention couples pixels that share h; the two branches share
  a joint softmax over the concatenated 256 keys. Scores are small enough
  that exp() stays in fp32 range without max-subtraction, so each branch
  independently produces an unnormalized output U = sum exp(s) * v and a
  partition function Z = sum exp(s); the joint softmax is (UH+UW)/(ZH+ZW).
  Z rides as an extra column appended to the V tile in the AV matmul; that
  column holds 1/gamma instead of 1, so Z' = Z/gamma and the final scale
  gamma/Z is just reciprocal(Z').

  Phase 1 loops over columns w in w-major pixel layout: projections, column
  scores ST[h',h] = K_w.T Q_w, est = exp(ST) * (1-I), UH|ZH' = est.T @
  [VT_w | 1/g]. VT and UH|ZH' are staged to DRAM in bf16. Phase 2 loops
  over rows h: strided-row DMA reads of the staged tensors perform the
  spatial transpose; the row branch accumulates UW|ZW' on top of the loaded
  UH|ZH' via an identity matmul into the same PSUM, and the final combine
  og = (U * recip(Z')) + (1+gamma*sigmoid(eca))*x is done in [w, c] layout.
  The host transposes [W,H,C] -> [C,H,W].

  Biases are folded into the projection matmuls via a ones-channel appended
  to x (channel index 288). Wq and Wk are packed into one [.., 100] weight
  (k at columns 64..100, gap zeroed) so q+k need one accumulation group.
  Loops are staged (all-projections, all-scores, all-AV per w-group) so the
  tensor engine sees dense back-to-back matmul runs and HAM stays warm.
"""

import sys

sys.path.insert(0, "/opt/trn_rl_repo")

import numpy as np
import ml_dtypes

B, C, H, W = 8, 288, 128, 128
CQ = 36
KOFF = 64          # k block starts at column/partition 64 of the packed qk
N_PIX = H * W
BF16 = ml_dtypes.bfloat16

GW = 16  # w-group size in phase 1 (DMA batching)
SW = 4   # qk projection subgroup (N = SW*128 = 512 per matmul)
GH = 16  # h-group size in phase 2

KCH = [(0, 128), (128, 128), (256, C + 1 - 256)]

_CACHE = {}


def _build_nc():
    import concourse.bass as bass
    import concourse.tile as tile
    import concourse.mybir as mybir
    from concourse import bacc
    from concourse.masks import make_identity

    f32 = mybir.dt.float32
    bf16 = mybir.dt.bfloat16
    AF = mybir.ActivationFunctionType

    class BaccSelfLoading(bacc.Bacc):
        # Keep matmuls self-loading (no standalone InstLdweights) so walrus
        # --enable-ldw-opt can schedule background weight-buffer loads.
        def move_matmul_waits_to_ldweights(self):
            pass

    nc = BaccSelfLoading()

    xw = nc.dram_tensor("xw", [C + 1, W, H], bf16, kind="ExternalInput")
    xt = nc.dram_tensor("xt", [W, H, C], bf16, kind="ExternalInput")
    wqkT = nc.dram_tensor("wqkT", [C + 1, 100], bf16, kind="ExternalInput")
    wvT = nc.dram_tensor("wvT", [C + 1, C], bf16, kind="ExternalInput")
    dmask = nc.dram_tensor("dmask", [H, H], bf16, kind="ExternalInput")
    eca = nc.dram_tensor("eca", [1, 3], f32, kind="ExternalInput")
    gam = nc.dram_tensor("gam", [1, 1], f32, kind="ExternalInput")
    out = nc.dram_tensor("out", [W, H, C], bf16, kind="ExternalOutput")

    with tile.TileContext(nc) as tc:
        with tc.tile_pool(name="persist", bufs=1) as persist, \
             tc.tile_pool(name="dram", bufs=1, space="DRAM") as dpool:
            vt_st = dpool.tile([H, W, C + 1], bf16)   # [h', w, c | 1/gamma]
            uh_st = dpool.tile([H, W, C + 1], bf16)   # [h(query), w, c | ZH']
            y_st = dpool.tile([1, 384], f32)
            fac_st = dpool.tile([1, C], f32)
            rgam_st = dpool.tile([1, 1], f32)

            q_sb = persist.tile([CQ, N_PIX], bf16)    # w-major: n = w*128 + h
            k_sb = persist.tile([CQ, N_PIX], bf16)
            ident = persist.tile([128, 128], f32)
            make_identity(nc, ident)
            identb = persist.tile([128, 128], bf16)
            nc.vector.tensor_copy(out=identb[:, :], in_=ident[:, :])
            mask_sb = persist.tile([H, H], bf16)
            nc.sync.dma_start(out=mask_sb[:, :], in_=dmask[:, :])

            wqk_sb = []
            wv_sb = []
            for (ofs, cnt) in KCH:
                t = persist.tile([cnt, 100], bf16, tag=f"wqk{ofs}")
                nc.sync.dma_start(out=t[:, :], in_=wqkT[ofs:ofs + cnt, :])
                wqk_sb.append(t)
                t = persist.tile([cnt, C], bf16, tag=f"wv{ofs}")
                nc.sync.dma_start(out=t[:, :], in_=wvT[ofs:ofs + cnt, :])
                wv_sb.append(t)

            eca_sb = persist.tile([1, 3], f32)
            nc.sync.dma_start(out=eca_sb[:, :], in_=eca[:, :])
            gam_sb = persist.tile([1, 1], f32)
            nc.sync.dma_start(out=gam_sb[:, :], in_=gam[:, :])
            rg_row = persist.tile([1, 1], f32)
            nc.vector.reciprocal(out=rg_row[:, :], in_=gam_sb[:, :])
            nc.sync.dma_start(out=rgam_st[:, :], in_=rg_row[:, :])
            rgcol = persist.tile([128, GW], bf16)
            nc.gpsimd.dma_start(
                out=rgcol[:, :].rearrange("p (w o) -> p w o", o=1),
                in_=rgam_st[0:1, 0:1].to_broadcast([128, GW, 1]),
            )
            fac_bc = persist.tile([128, C], f32)
            y_acc = persist.tile([128, 3], f32)
            nc.vector.memset(y_acc[:, :], 0.0)

            # ---------------- Phase 1: column branch (per w) ----------------
            with tc.tile_pool(name="p1x", bufs=2) as p1x, \
                 tc.tile_pool(name="p1g", bufs=2) as p1g, \
                 tc.tile_pool(name="p1s", bufs=18) as p1s, \
                 tc.tile_pool(name="p1r", bufs=4) as p1r, \
                 tc.tile_pool(name="qkp", bufs=1, space="PSUM") as qkp, \
                 tc.tile_pool(name="vtp", bufs=2, space="PSUM") as vtp, \
                 tc.tile_pool(name="stp", bufs=2, space="PSUM") as stp, \
                 tc.tile_pool(name="uhp", bufs=3, space="PSUM") as uhp:
                # HAM warmup: a dense burst of throwaway matmuls while the
                # first xw DMA is still in flight, to lift the PE clock gate
                # from 1.2 to 2.4 GHz before real work starts.
                warm_ps = stp.tile([128, 128], f32, tag="st")
                for _ in range(28):
                    nc.tensor.matmul(
                        warm_ps[:, :], wv_sb[0][:, 0:128], wv_sb[1][:, 0:128],
                        start=True, stop=True,
                    )
                for g in range(W // GW):
                    w0 = g * GW
                    xw_c = []
                    for j, (ofs, cnt) in enumerate(KCH):
                        t = p1x.tile([cnt, GW, H], bf16, tag=f"xw{j}")
                        nc.sync.dma_start(
                            out=t[:, :, :], in_=xw[ofs:ofs + cnt, w0:w0 + GW, :]
                        )
                        xw_c.append(t)

                    # q/k projections: one packed [.., 100] accumulation group
                    for s in range(GW // SW):
                        qk_ps = qkp.tile([100, SW * H], f32, tag="qkp")
                        for j, (ofs, cnt) in enumerate(KCH):
                            nc.tensor.matmul(
                                qk_ps[:, :], wqk_sb[j][:, :],
                                xw_c[j][:, s * SW:(s + 1) * SW, :],
                                start=(j == 0), stop=(j == len(KCH) - 1),
                            )
                        fo = (w0 + s * SW) * H
                        nc.vector.tensor_copy(
                            out=q_sb[:, fo:fo + SW * H], in_=qk_ps[0:CQ, :]
                        )
                        nc.scalar.copy(
                            out=k_sb[:, fo:fo + SW * H], in_=qk_ps[KOFF:KOFF + CQ, :]
                        )

                    # ECA channel-mean accumulation (skip the ones row);
                    # alternate DVE / ACT(accum_out) by group parity.
                    for j, (ofs, cnt) in enumerate(KCH):
                        rows = min(cnt, C - ofs)
                        part = p1r.tile([128, 1], f32, tag="red")
                        if g % 2 == 0:
                            nc.vector.reduce_sum(
                                out=part[:rows, :],
                                in_=xw_c[j][:rows, :, :],
                                axis=mybir.AxisListType.XY,
                            )
                        else:
                            scr = p1x.tile([cnt, GW, H], bf16, tag=f"scr{j}")
                            nc.scalar.activation(
                                out=scr[:rows, :, :], in_=xw_c[j][:rows, :, :],
                                func=AF.Copy, accum_out=part[:rows, :],
                            )
                        nc.vector.tensor_add(
                            out=y_acc[:rows, j:j + 1],
                            in0=y_acc[:rows, j:j + 1],
                            in1=part[:rows, :],
                        )

                    vtg = p1g.tile([128, GW, C + 1], bf16, tag="vtg")
                    uhg = p1g.tile([128, GW, C + 1], bf16, tag="uhg")
                    nc.vector.tensor_copy(
                        out=vtg[:, :, C:C + 1],
                        in_=rgcol[:, :].rearrange("p (w o) -> p w o", o=1),
                    )

                    # V-transpose tiles: VT_w [h', c] = x_w.T @ WvT
                    for wi in range(GW):
                        vt_ps = vtp.tile([128, C], f32, tag="vt")
                        for j, (ofs, cnt) in enumerate(KCH):
                            nc.tensor.matmul(
                                vt_ps[:, :], xw_c[j][:, wi, :], wv_sb[j][:, :],
                                start=(j == 0), stop=(j == len(KCH) - 1),
                            )
                        nc.vector.tensor_copy(out=vtg[:, wi, 0:C], in_=vt_ps[:, :])

                    # Column scores ST_w [h', h] = K_w.T @ Q_w, est = exp * mask
                    ests = []
                    for wi in range(GW):
                        fo = (w0 + wi) * H
                        st_ps = stp.tile([128, 128], f32, tag="st")
                        nc.tensor.matmul(
                            st_ps[:, :], k_sb[:, fo:fo + H], q_sb[:, fo:fo + H],
                            start=True, stop=True,
                        )
                        est = p1s.tile([128, 128], bf16, tag="est")
                        nc.scalar.activation(est[:, :], st_ps[:, :], AF.Exp)
                        nc.gpsimd.tensor_mul(
                            out=est[:, :], in0=est[:, :], in1=mask_sb[:, :]
                        )
                        ests.append(est)

                    # AV: UH_w [h, c | ZH'] = est.T @ [VT_w | 1/g]
                    for wi in range(GW):
                        uh_ps = uhp.tile([128, C + 1], f32, tag="uh")
                        nc.tensor.matmul(
                            uh_ps[:, :], ests[wi][:, :], vtg[:, wi, :],
                            start=True, stop=True,
                        )
                        nc.scalar.copy(out=uhg[:, wi, :], in_=uh_ps[:, :])

                    nc.gpsimd.dma_start(out=vt_st[:, w0:w0 + GW, :], in_=vtg[:, :, :])
                    nc.gpsimd.dma_start(out=uh_st[:, w0:w0 + GW, :], in_=uhg[:, :, :])

                # ---- interlude: ECA sigmoid factor ----
                nc.sync.dma_start(
                    out=y_st[0:1, :].rearrange("o (j p) -> (o p) j", p=128),
                    in_=y_acc[:, :],
                )
                y_row = p1r.tile([1, 292], f32, tag="yrow")
                nc.vector.memset(y_row[:, :], 0.0)
                nc.sync.dma_start(out=y_row[0:1, 1:C + 1], in_=y_st[0:1, 0:C])
                nc.scalar.mul(y_row[:, 1:C + 1], y_row[:, 1:C + 1], 1.0 / N_PIX)
                yc = p1r.tile([1, C], f32, tag="yc")
                tmp = p1r.tile([1, C], f32, tag="yt")
                nc.vector.tensor_scalar_mul(
                    out=yc[:, :], in0=y_row[:, 0:C], scalar1=eca_sb[:, 0:1]
                )
                nc.vector.tensor_scalar_mul(
                    out=tmp[:, :], in0=y_row[:, 1:C + 1], scalar1=eca_sb[:, 1:2]
                )
                nc.vector.tensor_add(out=yc[:, :], in0=yc[:, :], in1=tmp[:, :])
                nc.vector.tensor_scalar_mul(
                    out=tmp[:, :], in0=y_row[:, 2:C + 2], scalar1=eca_sb[:, 2:3]
                )
                nc.vector.tensor_add(out=yc[:, :], in0=yc[:, :], in1=tmp[:, :])
                nc.scalar.activation(yc[:, :], yc[:, :], AF.Sigmoid)
                nc.vector.tensor_scalar_mul(
                    out=yc[:, :], in0=yc[:, :], scalar1=gam_sb[0:1, 0:1]
                )
                nc.vector.tensor_scalar_add(out=yc[:, :], in0=yc[:, :], scalar1=1.0)
                nc.sync.dma_start(out=fac_st[:, :], in_=yc[:, :])
                nc.gpsimd.dma_start(
                    out=fac_bc[:, :], in_=fac_st[0:1, :].to_broadcast([128, C])
                )

            # ---------------- Phase 2: row branch + combine (per h) ---------
            q_v = q_sb[:, :].rearrange("p (w h) -> p h w", h=H)
            k_v = k_sb[:, :].rearrange("p (w h) -> p h w", h=H)
            with tc.tile_pool(name="p2b", bufs=2) as p2b, \
                 tc.tile_pool(name="p2s", bufs=4) as p2s, \
                 tc.tile_pool(name="p2r", bufs=8) as p2r, \
                 tc.tile_pool(name="stp2", bufs=3, space="PSUM") as stp2, \
                 tc.tile_pool(name="uwp", bufs=3, space="PSUM") as uwp:
                for g in range(H // GH):
                    h0 = g * GH
                    vtr = p2b.tile([W, GH, C + 1], bf16, tag="vtr")
                    nc.sync.dma_start(
                        out=vtr[:, :, :],
                        in_=vt_st[h0:h0 + GH, :, :].rearrange("h w c -> w h c"),
                    )
                    uhr = p2b.tile([W, GH, C + 1], bf16, tag="uhr")
                    nc.sync.dma_start(
                        out=uhr[:, :, :],
                        in_=uh_st[h0:h0 + GH, :, :].rearrange("h w c -> w h c"),
                    )
                    xtr = p2b.tile([W, GH, C], bf16, tag="xtr")
                    nc.sync.dma_start(out=xtr[:, :, :], in_=xt[:, h0:h0 + GH, :])
                    og = p2b.tile([W, GH, C], bf16, tag="og")

                    for hi in range(GH):
                        h = h0 + hi
                        stw_ps = stp2.tile([128, 128], f32, tag="stw")
                        nc.tensor.matmul(
                            stw_ps[:, :], k_v[:, h, :], q_v[:, h, :],
                            start=True, stop=True,
                        )
                        estw = p2s.tile([128, 128], bf16, tag="estw")
                        nc.scalar.activation(estw[:, :], stw_ps[:, :], AF.Exp)

                        # UW|ZW' then accumulate the loaded UH|ZH' via identity
                        uw_ps = uwp.tile([128, C + 1], f32, tag="uw")
                        nc.tensor.matmul(
                            uw_ps[:, :], estw[:, :], vtr[:, hi, :],
                            start=True, stop=False,
                        )
                        nc.tensor.matmul(
                            uw_ps[:, :], identb[:, :], uhr[:, hi, :],
                            start=False, stop=True,
                        )

                        rz = p2r.tile([128, 1], f32, tag="rz")
                        nc.vector.reciprocal(out=rz[:, :], in_=uw_ps[:, C:C + 1])
                        xtmp = p2s.tile([128, C], f32, tag="xtmp")
                        nc.gpsimd.tensor_mul(
                            out=xtmp[:, :], in0=xtr[:, hi, :], in1=fac_bc[:, :]
                        )
                        # og = (UH+UW) * (gamma/Z) + fac*x  in one DVE op
                        nc.vector.scalar_tensor_tensor(
                            out=og[:, hi, :], in0=uw_ps[:, 0:C], scalar=rz[:, :],
                            in1=xtmp[:, :],
                            op0=mybir.AluOpType.mult, op1=mybir.AluOpType.add,
                        )

                    nc.gpsimd.dma_start(out=out[:, h0:h0 + GH, :], in_=og[:, :, :])

    nc.compile()
    _fuse_ldweights(nc, mybir)
    return nc


def _fuse_ldweights(nc, mybir):
    """Fold standalone InstLdweights back into their (self-loading) matmuls.

    Tile splits every non-f32 matmul into InstLdweights + InstMatmult; that
    form is rejected by walrus --enable-ldw-opt, which schedules weight loads
    into the background weight buffer (so LDWEIGHTS overlaps the previous
    matmul instead of serializing with it). The InstMatmult still carries the
    weights operand, so the standalone load is redundant: merge its waits
    into the matmul (spilling to an event-semaphore nop when both carry a
    wait, since most instructions may hold only one) and drop it.
    """
    import bass_rust

    fn = nc.m.functions[0]
    for blk in fn.blocks:
        insts = blk.instructions
        new = []
        pending = []
        changed = False
        for ins in insts:
            tn = type(ins).__name__
            if tn == "InstLdweights":
                pending.append(ins)
                changed = True
                continue
            if tn == "InstMatmult" and pending:
                ld = pending.pop(0)
                lsi = ld.sync_info
                msi = ins.sync_info
                lw = list(lsi.on_wait) if lsi else []
                lu = list(lsi.on_update) if lsi else []
                mw = list(msi.on_wait) if msi else []
                mu = list(msi.on_update) if msi else []
                ins.ldweights = True
                if lw and mw:
                    ev = mybir.InstEventSemaphore(
                        name=f"EVL-{ld.name}", engine=ld.engine,
                        sync_info=bass_rust.SyncInfo(on_wait=lw, on_update=[]),
                        ins=[], outs=[],
                    )
                    new.append(ev)
                    lw = []
                ins.sync_info = bass_rust.SyncInfo(
                    on_wait=lw + mw, on_update=lu + mu
                )
            new.append(ins)
        assert not pending, f"unpaired InstLdweights in {blk.name}"
        if changed:
            blk.instructions = new


def _get_nc():
    if "nc" not in _CACHE:
        _CACHE["nc"] = _build_nc()
    return _CACHE["nc"]


def _prep_inputs(x, Wq, bq, Wk, bk, Wv, bv, eca_w, gamma):
    x = np.asarray(x, np.float32)
    wqk = np.zeros((C + 1, 100), np.float32)
    wqk[0:C, 0:CQ] = np.asarray(Wq, np.float32).T
    wqk[C, 0:CQ] = np.asarray(bq, np.float32)
    wqk[0:C, KOFF:KOFF + CQ] = np.asarray(Wk, np.float32).T
    wqk[C, KOFF:KOFF + CQ] = np.asarray(bk, np.float32)
    wqkT = wqk.astype(BF16)
    wvT = np.concatenate([np.asarray(Wv, np.float32).T,
                          np.asarray(bv, np.float32)[None, :]]).astype(BF16)
    dmask = (1.0 - np.eye(H, dtype=np.float32)).astype(BF16)
    eca = np.asarray(eca_w, np.float32).reshape(1, 3)
    gam = np.asarray(gamma, np.float32).reshape(1, 1)

    ones_plane = np.ones((1, W, H), np.float32)
    in_maps = []
    for b in range(B):
        xb = x[b]                                           # [c, h, w]
        xwv = np.ascontiguousarray(xb.transpose(0, 2, 1))   # [c, w, h]
        xwv = np.concatenate([xwv, ones_plane]).astype(BF16)
        xtv = np.ascontiguousarray(xb.transpose(2, 1, 0)).astype(BF16)  # [w,h,c]
        in_maps.append({
            "xw": xwv, "xt": xtv, "wqkT": wqkT, "wvT": wvT,
            "dmask": dmask, "eca": eca, "gam": gam,
        })
    return in_maps


def _enable_ldw_opt():
    """Compile with walrus --enable-ldw-opt=true: background weight-buffer
    loads let LDWEIGHTS overlap in-flight matmuls (concourse defaults it off).
    """
    import concourse.bass_utils as bu
    if getattr(bu.run_command, "_ldw_patched", False):
        return
    orig = bu.run_command

    def run_command_ldw(argv, **kw):
        argv = [a.replace("--enable-ldw-opt=false", "--enable-ldw-opt=true")
                if isinstance(a, str) else a for a in argv]
        return orig(argv, **kw)

    run_command_ldw._ldw_patched = True
    bu.run_command = run_command_ldw


def kernel(x, Wq, bq, Wk, bk, Wv, bv, eca_w, gamma, _return_results=False,
           **run_kwargs):
    from concourse.bass_utils import run_bass_kernel_spmd

    _enable_ldw_opt()
    nc = _get_nc()
    in_maps = _prep_inputs(x, Wq, bq, Wk, bk, Wv, bv, eca_w, gamma)
    res = run_bass_kernel_spmd(nc, in_maps, core_ids=list(range(B)), **run_kwargs)
    out = np.empty((B, C, H, W), np.float32)
    for b in range(B):
        # device output is [w, h, c]
        out[b] = res.results[b]["out"].astype(np.float32).transpose(2, 1, 0)
    if _return_results:
        return out, res
    return out


# revision 16
# speedup vs baseline: 1.0120x; 1.0120x over previous
"""Criss-cross attention (CCAttention) Trainium2 kernel.

Shapes (hardcoded): x [8, 288, 128, 128] f32, Wq/Wk [36, 288], Wv [288, 288],
bq/bk [36], bv [288], eca_w [3], gamma [1]. Output [8, 288, 128, 128] f32.

Sharding: pure data parallel — one batch element per NeuronCore (8 cores).

Per-core algorithm (batch index dropped):
  q/k/v are 1x1 convs (channel GEMMs). Column attention couples pixels that
  share w; row attention couples pixels that share h; the two branches share
  a joint softmax over the concatenated 256 keys. Scores are small enough
  that exp() stays in fp32 range without max-subtraction, so each branch
  independently produces an unnormalized output U = sum exp(s) * v and a
  partition function Z = sum exp(s); the joint softmax is (UH+UW)/(ZH+ZW).
  Z rides as an extra column appended to the V tile in the AV matmul; that
  column holds 1/gamma instead of 1, so Z' = Z/gamma and the final scale
  gamma/Z is just reciprocal(Z').

  Phase 1 loops over columns w in w-major pixel layout: projections, column
  scores ST[h',h] = K_w.T Q_w, est = exp(ST) * (1-I), UH|ZH' = est.T @
  [VT_w | 1/g]. VT and UH|ZH' are staged to DRAM in bf16. Phase 2 loops
  over rows h: strided-row DMA reads of the staged tensors perform the
  spatial transpose; the row branch accumulates UW|ZW' on top of the loaded
  UH|ZH' via an identity matmul into the same PSUM, and the final combine
  og = (U * recip(Z')) + (1+gamma*sigmoid(eca))*x is done in [w, c] layout.
  The host transposes [W,H,C] -> [C,H,W].

  Biases are folded into the projection matmuls via a ones-channel appended
  to x (channel index 288). Wq and Wk are packed into one [.., 100] weight
  (k at columns 64..100, gap zeroed) so q+k need one accumulation group.
  Loops are staged (all-projections, all-scores, all-AV per w-group) so the
  tensor engine sees dense back-to-back matmul runs and HAM stays warm.
"""

import sys

sys.path.insert(0, "/opt/trn_rl_repo")

import numpy as np
import ml_dtypes

B, C, H, W = 8, 288, 128, 128
CQ = 36
KOFF = 64          # k block starts at column/partition 64 of the packed qk
N_PIX = H * W
BF16 = ml_dtypes.bfloat16

GW = 16  # w-group size in phase 1 (DMA batching)
SW = 4   # qk projection subgroup (N = SW*128 = 512 per matmul)
GH = 16  # h-group size in phase 2

KCH = [(0, 128), (128, 128), (256, C + 1 - 256)]

_CACHE = {}


def _build_nc():
    import concourse.bass as bass
    import concourse.tile as tile
    import concourse.mybir as mybir
    from concourse import bacc
    from concourse.masks import make_identity

    f32 = mybir.dt.float32
    bf16 = mybir.dt.bfloat16
    AF = mybir.ActivationFunctionType

    nc = bacc.Bacc()

    xw = nc.dram_tensor("xw", [C + 1, W, H], bf16, kind="ExternalInput")
    xt = nc.dram_tensor("xt", [W, H, C], bf16, kind="ExternalInput")
    wqkT = nc.dram_tensor("wqkT", [C + 1, 100], bf16, kind="ExternalInput")
    wvT = nc.dram_tensor("wvT", [C + 1, C], bf16, kind="ExternalInput")
    dmask = nc.dram_tensor("dmask", [H, H], bf16, kind="ExternalInput")
    eca = nc.dram_tensor("eca", [1, 3], f32, kind="ExternalInput")
    gam = nc.dram_tensor("gam", [1, 1], f32, kind="ExternalInput")
    out = nc.dram_tensor("out", [W, H, C], bf16, kind="ExternalOutput")

    with tile.TileContext(nc) as tc:
        with tc.tile_pool(name="persist", bufs=1) as persist, \
             tc.tile_pool(name="dram", bufs=1, space="DRAM") as dpool:
            vt_st = dpool.tile([H, W, C + 1], bf16)   # [h', w, c | 1/gamma]
            uh_st = dpool.tile([H, W, C + 1], bf16)   # [h(query), w, c | ZH']
            y_st = dpool.tile([1, 384], f32)
            fac_st = dpool.tile([1, C], f32)
            rgam_st = dpool.tile([1, 1], f32)

            q_sb = persist.tile([CQ, N_PIX], bf16)    # w-major: n = w*128 + h
            k_sb = persist.tile([CQ, N_PIX], bf16)
            ident = persist.tile([128, 128], f32)
            make_identity(nc, ident)
            identb = persist.tile([128, 128], bf16)
            nc.vector.tensor_copy(out=identb[:, :], in_=ident[:, :])
            mask_sb = persist.tile([H, H], bf16)
            nc.sync.dma_start(out=mask_sb[:, :], in_=dmask[:, :])

            wqk_sb = []
            wv_sb = []
            for (ofs, cnt) in KCH:
                t = persist.tile([cnt, 100], bf16, tag=f"wqk{ofs}")
                nc.sync.dma_start(out=t[:, :], in_=wqkT[ofs:ofs + cnt, :])
                wqk_sb.append(t)
                t = persist.tile([cnt, C], bf16, tag=f"wv{ofs}")
                nc.sync.dma_start(out=t[:, :], in_=wvT[ofs:ofs + cnt, :])
                wv_sb.append(t)

            eca_sb = persist.tile([1, 3], f32)
            nc.sync.dma_start(out=eca_sb[:, :], in_=eca[:, :])
            gam_sb = persist.tile([1, 1], f32)
            nc.sync.dma_start(out=gam_sb[:, :], in_=gam[:, :])
            rg_row = persist.tile([1, 1], f32)
            nc.vector.reciprocal(out=rg_row[:, :], in_=gam_sb[:, :])
            nc.sync.dma_start(out=rgam_st[:, :], in_=rg_row[:, :])
            rgcol = persist.tile([128, GW], bf16)
            nc.gpsimd.dma_start(
                out=rgcol[:, :].rearrange("p (w o) -> p w o", o=1),
                in_=rgam_st[0:1, 0:1].to_broadcast([128, GW, 1]),
            )
            fac_bc = persist.tile([128, C], f32)
            y_acc = persist.tile([128, 3], f32)
            nc.vector.memset(y_acc[:, :], 0.0)

            # ---------------- Phase 1: column branch (per w) ----------------
            with tc.tile_pool(name="p1x", bufs=2) as p1x, \
                 tc.tile_pool(name="p1g", bufs=2) as p1g, \
                 tc.tile_pool(name="p1s", bufs=18) as p1s, \
                 tc.tile_pool(name="p1r", bufs=4) as p1r, \
                 tc.tile_pool(name="qkp", bufs=1, space="PSUM") as qkp, \
                 tc.tile_pool(name="vtp", bufs=2, space="PSUM") as vtp, \
                 tc.tile_pool(name="stp", bufs=3, space="PSUM") as stp, \
                 tc.tile_pool(name="uhp", bufs=2, space="PSUM") as uhp:
                for g in range(W // GW):
                    w0 = g * GW
                    xw_c = []
                    for j, (ofs, cnt) in enumerate(KCH):
                        t = p1x.tile([cnt, GW, H], bf16, tag=f"xw{j}")
                        nc.sync.dma_start(
                            out=t[:, :, :], in_=xw[ofs:ofs + cnt, w0:w0 + GW, :]
                        )
                        xw_c.append(t)

                    # q/k projections: one packed [.., 100] accumulation group
                    for s in range(GW // SW):
                        qk_ps = qkp.tile([100, SW * H], f32, tag="qkp")
                        for j, (ofs, cnt) in enumerate(KCH):
                            nc.tensor.matmul(
                                qk_ps[:, :], wqk_sb[j][:, :],
                                xw_c[j][:, s * SW:(s + 1) * SW, :],
                                start=(j == 0), stop=(j == len(KCH) - 1),
                            )
                        fo = (w0 + s * SW) * H
                        nc.vector.tensor_copy(
                            out=q_sb[:, fo:fo + SW * H], in_=qk_ps[0:CQ, :]
                        )
                        nc.scalar.copy(
                            out=k_sb[:, fo:fo + SW * H], in_=qk_ps[KOFF:KOFF + CQ, :]
                        )

                    # ECA channel-mean accumulation (skip the ones row);
                    # alternate DVE / ACT(accum_out) by group parity.
                    for j, (ofs, cnt) in enumerate(KCH):
                        rows = min(cnt, C - ofs)
                        part = p1r.tile([128, 1], f32, tag="red")
                        if g % 2 == 0:
                            nc.vector.reduce_sum(
                                out=part[:rows, :],
                                in_=xw_c[j][:rows, :, :],
                                axis=mybir.AxisListType.XY,
                            )
                        else:
                            scr = p1x.tile([cnt, GW, H], bf16, tag=f"scr{j}")
                            nc.scalar.activation(
                                out=scr[:rows, :, :], in_=xw_c[j][:rows, :, :],
                                func=AF.Copy, accum_out=part[:rows, :],
                            )
                        nc.vector.tensor_add(
                            out=y_acc[:rows, j:j + 1],
                            in0=y_acc[:rows, j:j + 1],
                            in1=part[:rows, :],
                        )

                    vtg = p1g.tile([128, GW, C + 1], bf16, tag="vtg")
                    uhg = p1g.tile([128, GW, C + 1], bf16, tag="uhg")
                    nc.vector.tensor_copy(
                        out=vtg[:, :, C:C + 1],
                        in_=rgcol[:, :].rearrange("p (w o) -> p w o", o=1),
                    )

                    # V-transpose tiles: VT_w [h', c] = x_w.T @ WvT
                    for wi in range(GW):
                        vt_ps = vtp.tile([128, C], f32, tag="vt")
                        for j, (ofs, cnt) in enumerate(KCH):
                            nc.tensor.matmul(
                                vt_ps[:, :], xw_c[j][:, wi, :], wv_sb[j][:, :],
                                start=(j == 0), stop=(j == len(KCH) - 1),
                            )
                        nc.vector.tensor_copy(out=vtg[:, wi, 0:C], in_=vt_ps[:, :])

                    # Column scores ST_w [h', h] = K_w.T @ Q_w, est = exp * mask
                    ests = []
                    for wi in range(GW):
                        fo = (w0 + wi) * H
                        st_ps = stp.tile([128, 128], f32, tag="st")
                        nc.tensor.matmul(
                            st_ps[:, :], k_sb[:, fo:fo + H], q_sb[:, fo:fo + H],
                            start=True, stop=True,
                        )
                        est = p1s.tile([128, 128], bf16, tag="est")
                        nc.scalar.activation(est[:, :], st_ps[:, :], AF.Exp)
                        nc.gpsimd.tensor_mul(
                            out=est[:, :], in0=est[:, :], in1=mask_sb[:, :]
                        )
                        ests.append(est)

                    # AV: UH_w [h, c | ZH'] = est.T @ [VT_w | 1/g]
                    for wi in range(GW):
                        uh_ps = uhp.tile([128, C + 1], f32, tag="uh")
                        nc.tensor.matmul(
                            uh_ps[:, :], ests[wi][:, :], vtg[:, wi, :],
                            start=True, stop=True,
                        )
                        nc.scalar.copy(out=uhg[:, wi, :], in_=uh_ps[:, :])

                    nc.gpsimd.dma_start(out=vt_st[:, w0:w0 + GW, :], in_=vtg[:, :, :])
                    nc.gpsimd.dma_start(out=uh_st[:, w0:w0 + GW, :], in_=uhg[:, :, :])

                # ---- interlude: ECA sigmoid factor ----
                nc.sync.dma_start(
                    out=y_st[0:1, :].rearrange("o (j p) -> (o p) j", p=128),
                    in_=y_acc[:, :],
                )
                y_row = p1r.tile([1, 292], f32, tag="yrow")
                nc.vector.memset(y_row[:, :], 0.0)
                nc.sync.dma_start(out=y_row[0:1, 1:C + 1], in_=y_st[0:1, 0:C])
                nc.scalar.mul(y_row[:, 1:C + 1], y_row[:, 1:C + 1], 1.0 / N_PIX)
                yc = p1r.tile([1, C], f32, tag="yc")
                tmp = p1r.tile([1, C], f32, tag="yt")
                nc.vector.tensor_scalar_mul(
                    out=yc[:, :], in0=y_row[:, 0:C], scalar1=eca_sb[:, 0:1]
                )
                nc.vector.tensor_scalar_mul(
                    out=tmp[:, :], in0=y_row[:, 1:C + 1], scalar1=eca_sb[:, 1:2]
                )
                nc.vector.tensor_add(out=yc[:, :], in0=yc[:, :], in1=tmp[:, :])
                nc.vector.tensor_scalar_mul(
                    out=tmp[:, :], in0=y_row[:, 2:C + 2], scalar1=eca_sb[:, 2:3]
                )
                nc.vector.tensor_add(out=yc[:, :], in0=yc[:, :], in1=tmp[:, :])
                nc.scalar.activation(yc[:, :], yc[:, :], AF.Sigmoid)
                nc.vector.tensor_scalar_mul(
                    out=yc[:, :], in0=yc[:, :], scalar1=gam_sb[0:1, 0:1]
                )
                nc.vector.tensor_scalar_add(out=yc[:, :], in0=yc[:, :], scalar1=1.0)
                nc.sync.dma_start(out=fac_st[:, :], in_=yc[:, :])
                nc.gpsimd.dma_start(
                    out=fac_bc[:, :], in_=fac_st[0:1, :].to_broadcast([128, C])
                )

            # ---------------- Phase 2: row branch + combine (per h) ---------
            q_v = q_sb[:, :].rearrange("p (w h) -> p h w", h=H)
            k_v = k_sb[:, :].rearrange("p (w h) -> p h w", h=H)
            with tc.tile_pool(name="p2b", bufs=2) as p2b, \
                 tc.tile_pool(name="p2s", bufs=4) as p2s, \
                 tc.tile_pool(name="p2r", bufs=8) as p2r, \
                 tc.tile_pool(name="stp2", bufs=3, space="PSUM") as stp2, \
                 tc.tile_pool(name="uwp", bufs=3, space="PSUM") as uwp:
                for g in range(H // GH):
                    h0 = g * GH
                    vtr = p2b.tile([W, GH, C + 1], bf16, tag="vtr")
                    nc.sync.dma_start(
                        out=vtr[:, :, :],
                        in_=vt_st[h0:h0 + GH, :, :].rearrange("h w c -> w h c"),
                    )
                    uhr = p2b.tile([W, GH, C + 1], bf16, tag="uhr")
                    nc.sync.dma_start(
                        out=uhr[:, :, :],
                        in_=uh_st[h0:h0 + GH, :, :].rearrange("h w c -> w h c"),
                    )
                    xtr = p2b.tile([W, GH, C], bf16, tag="xtr")
                    nc.sync.dma_start(out=xtr[:, :, :], in_=xt[:, h0:h0 + GH, :])
                    og = p2b.tile([W, GH, C], bf16, tag="og")

                    for hi in range(GH):
                        h = h0 + hi
                        stw_ps = stp2.tile([128, 128], f32, tag="stw")
                        nc.tensor.matmul(
                            stw_ps[:, :], k_v[:, h, :], q_v[:, h, :],
                            start=True, stop=True,
                        )
                        estw = p2s.tile([128, 128], bf16, tag="estw")
                        nc.scalar.activation(estw[:, :], stw_ps[:, :], AF.Exp)

                        # UW|ZW' then accumulate the loaded UH|ZH' via identity
                        uw_ps = uwp.tile([128, C + 1], f32, tag="uw")
                        nc.tensor.matmul(
                            uw_ps[:, :], estw[:, :], vtr[:, hi, :],
                            start=True, stop=False,
                        )
                        nc.tensor.matmul(
                            uw_ps[:, :], identb[:, :], uhr[:, hi, :],
                            start=False, stop=True,
                        )

                        rz = p2r.tile([128, 1], f32, tag="rz")
                        nc.vector.reciprocal(out=rz[:, :], in_=uw_ps[:, C:C + 1])
                        xtmp = p2s.tile([128, C], f32, tag="xtmp")
                        nc.gpsimd.tensor_mul(
                            out=xtmp[:, :], in0=xtr[:, hi, :], in1=fac_bc[:, :]
                        )
                        # og = (UH+UW) * (gamma/Z) + fac*x  in one DVE op
                        nc.vector.scalar_tensor_tensor(
                            out=og[:, hi, :], in0=uw_ps[:, 0:C], scalar=rz[:, :],
                            in1=xtmp[:, :],
                            op0=mybir.AluOpType.mult, op1=mybir.AluOpType.add,
                        )

                    nc.gpsimd.dma_start(out=out[:, h0:h0 + GH, :], in_=og[:, :, :])

    nc.compile()
    return nc


def _get_nc():
    if "nc" not in _CACHE:
        _CACHE["nc"] = _build_nc()
    return _CACHE["nc"]


def _prep_inputs(x, Wq, bq, Wk, bk, Wv, bv, eca_w, gamma):
    x = np.asarray(x, np.float32)
    wqk = np.zeros((C + 1, 100), np.float32)
    wqk[0:C, 0:CQ] = np.asarray(Wq, np.float32).T
    wqk[C, 0:CQ] = np.asarray(bq, np.float32)
    wqk[0:C, KOFF:KOFF + CQ] = np.asarray(Wk, np.float32).T
    wqk[C, KOFF:KOFF + CQ] = np.asarray(bk, np.float32)
    wqkT = wqk.astype(BF16)
    wvT = np.concatenate([np.asarray(Wv, np.float32).T,
                          np.asarray(bv, np.float32)[None, :]]).astype(BF16)
    dmask = (1.0 - np.eye(H, dtype=np.float32)).astype(BF16)
    eca = np.asarray(eca_w, np.float32).reshape(1, 3)
    gam = np.asarray(gamma, np.float32).reshape(1, 1)

    ones_plane = np.ones((1, W, H), np.float32)
    in_maps = []
    for b in range(B):
        xb = x[b]                                           # [c, h, w]
        xwv = np.ascontiguousarray(xb.transpose(0, 2, 1))   # [c, w, h]
        xwv = np.concatenate([xwv, ones_plane]).astype(BF16)
        xtv = np.ascontiguousarray(xb.transpose(2, 1, 0)).astype(BF16)  # [w,h,c]
        in_maps.append({
            "xw": xwv, "xt": xtv, "wqkT": wqkT, "wvT": wvT,
            "dmask": dmask, "eca": eca, "gam": gam,
        })
    return in_maps


def kernel(x, Wq, bq, Wk, bk, Wv, bv, eca_w, gamma, _return_results=False,
           **run_kwargs):
    from concourse.bass_utils import run_bass_kernel_spmd

    nc = _get_nc()
    in_maps = _prep_inputs(x, Wq, bq, Wk, bk, Wv, bv, eca_w, gamma)
    res = run_bass_kernel_spmd(nc, in_maps, core_ids=list(range(B)), **run_kwargs)
    out = np.empty((B, C, H, W), np.float32)
    for b in range(B):
        # device output is [w, h, c]
        out[b] = res.results[b]["out"].astype(np.float32).transpose(2, 1, 0)
    if _return_results:
        return out, res
    return out
